# revision 31
# baseline (speedup 1.0000x reference)
"""FOFEReader Trainium2 kernel: 8-core SPMD (batch x s-half sharding), v2.

Math (per batch b, candidate (s, e=s+j), j<16):
  F[t] = sum_{k<=t} a^(t-k) doc[k]   (prefix FOFE),  R[t] = sum_{k>=t} a^(k-t) doc[k]
  x = [F[s-1] | F[s+j] - a^(j+1) F[s-1] | R[s+j+1] | qf]
  out = (relu(bn2(relu(bn1(x @ W1.T)) @ W2.T)) @ W3.T)
Reformulated so the 1212-dim GEMM is shared across the 16 spans j:
  G_u = U_u @ F (u in {l,c}), G_r = U_r @ R   with W1.T = [U_l U_c U_r U_q] row blocks
  z[s,j] = (G_l[s-1] + q1)' + (G_c[s+j] + G_r[s+j+1])' - a^(j+1) G_c[s-1]'
where ' marks the bn1 scale folded in at PSUM eviction.

v2 structure (vs v1):
  - bf16 for the L2/L3 matmul operands (fp16 moving runs ~1.2x slower on PE);
    z assembly stays fp16 for mantissa.
  - E = Gc[t] + Gr[t+1] accumulated IN PSUM via a shifted matmul output AP
    (one eviction instead of two + a vector add).
  - q-path computed on host, shipped via bn1x (kills ~30 device instructions).
  - batched DVE assembly: one TS (4x mode) + two TT (2x mode) over [128,8,406]
    per span instead of 16 per-tile ops; one ScalarE relu per span.
  - L2 runs 4 spans per weight load (jblk=4) into 2-bank PSUM tiles; h2
    eviction is one activation per (mt, span-pair).
  - warmup matmul bursts keep the PE HAM clock-gate at 8/8 from ~4us.
  - single straight DMA per input tensor (host pre-packs SBUF layouts).
"""
import os
import sys

for _p in ("/opt/trn_rl_repo", "/root/.axon_site/_ro/trn_rl_repo"):
    if os.path.isdir(_p) and _p not in sys.path:
        sys.path.insert(0, _p)
        break

import numpy as np

T = 809
MSPAN = 16
B = 4
ALPHA = 0.9
NS = 406          # s-starts per core
WIN = 424         # t window per core: t = s_lo-1 + i, i in [0, 424)
DD = 304
EMB = 300
LQ = 30
H4 = 1024
H2 = 512
BN_EPS = 1e-5
N_CORES = 8
NHT = H4 // 128   # 8
NMT = H2 // 128   # 4
NWARM_HEAD = 44
NWARM_MID = 30
EPAD = 432        # padded E row length (keeps slices 2-byte packed)

_CACHE = {}

KT1 = [(0, 128), (128, 128), (256, 48)]       # d-tiles of 304
NKDOC = 5                                     # 640-row per-core token window


def _build_amat(s_lo):
    """[809, 848] fp32: cols 0..423 = forward-FOFE operator columns for
    t=s_lo-1+i (A^T slice), cols 424.. = reverse. Out-of-range t -> zero col."""
    t_idx = s_lo - 1 + np.arange(WIN)
    kv = np.arange(T)[:, None]
    tv = t_idx[None, :]
    valid = ((t_idx >= 0) & (t_idx <= T - 1))[None, :]
    af = np.where((kv <= tv) & valid, ALPHA ** np.maximum(tv - kv, 0), 0.0)
    ar = np.where((kv >= tv) & valid, ALPHA ** np.maximum(kv - tv, 0), 0.0)
    return np.concatenate([af, ar], axis=1).astype(np.float32)


def _cand_indices():
    s_list, e_list = [], []
    for s in range(T):
        for span in range(min(MSPAN, T - s)):
            s_list.append(s)
            e_list.append(s + span)
    return np.asarray(s_list, np.int64), np.asarray(e_list, np.int64)


def _pack5(a, width, w0):
    """rows [w0, w0+640) of [809, width] -> [128, 5, width] partition-major.
    FOFE decay makes rows outside the window contribute < 1e-9 relative."""
    out = np.ascontiguousarray(a[w0:w0 + 640])
    return np.ascontiguousarray(out.reshape(5, 128, width).transpose(1, 0, 2))


def _build_bass():
    import concourse.bacc as bacc
    import concourse.tile as tile
    from concourse import mybir
    from contextlib import ExitStack

    F32 = mybir.dt.float32
    F16 = mybir.dt.float16
    BF16 = mybir.dt.bfloat16
    F8 = mybir.dt.float8e4
    DR = mybir.MatmulPerfMode.DoubleRow
    AF = mybir.ActivationFunctionType
    OP = mybir.AluOpType

    nc = bacc.Bacc("TRN2", target_bir_lowering=False, debug=False,
                   num_devices=N_CORES)

    doc = nc.dram_tensor("doc", [128, NKDOC, DD], BF16, kind="ExternalInput").ap()
    amat = nc.dram_tensor("amat", [128, NKDOC, 2 * WIN], BF16, kind="ExternalInput").ap()
    w1 = nc.dram_tensor("w1", [128, 6, H4], BF16, kind="ExternalInput").ap()
    w1s = nc.dram_tensor("w1s", [48, 3, H4], BF16, kind="ExternalInput").ap()
    w2 = nc.dram_tensor("w2", [128, 4, H2], BF16, kind="ExternalInput").ap()
    w2f8 = nc.dram_tensor("w2f8", [128, 2, 2, H2], F8, kind="ExternalInput").ap()
    w3 = nc.dram_tensor("w3", [128, NMT, 2], BF16, kind="ExternalInput").ap()
    bn1x = nc.dram_tensor("bn1x", [128, 2, NHT], F32, kind="ExternalInput").ap()
    bn2x = nc.dram_tensor("bn2x", [128, 2, NMT], F32, kind="ExternalInput").ap()
    y = nc.dram_tensor("y", [2, MSPAN, NS], F32, kind="ExternalOutput").ap()

    with ExitStack() as ctx:
        tc = ctx.enter_context(tile.TileContext(nc))
        const = ctx.enter_context(tc.tile_pool(name="const", bufs=1))
        work = ctx.enter_context(tc.tile_pool(name="work", bufs=2))
        h1p = ctx.enter_context(tc.tile_pool(name="h1p", bufs=8))
        h2p = ctx.enter_context(tc.tile_pool(name="h2p", bufs=6))
        yp = ctx.enter_context(tc.tile_pool(name="yp", bufs=3))

        # ---- scratch for warmup (no DMA dependency) ----
        scratch = const.tile([128, WIN], BF16, tag="scratch")
        nc.vector.memset(scratch, 0.01)

        # ---- chunked DMAs: transfers start early and pipeline with use ----
        # sync queue: amat (chunks, F/R critical path) then w2
        # gpsimd queue: bn, doc, w1 (chunks), w3
        amat_sb = const.tile([128, NKDOC, 2 * WIN], BF16, tag="amat")
        for c0, c1 in ((0, 2), (2, 4), (4, NKDOC)):
            nc.sync.dma_start(out=amat_sb[:, c0:c1, :], in_=amat[:, c0:c1, :])
        w2_sb = const.tile([128, 4, H2], BF16, tag="w2")
        for c0, c1 in ((0, 2), (2, 4)):
            nc.sync.dma_start(out=w2_sb[:, c0:c1, :], in_=w2[:, c0:c1, :])
        w2f8_sb = const.tile([128, 2, 2, H2], F8, tag="w2f8")
        nc.sync.dma_start(out=w2f8_sb, in_=w2f8)
        bn1_sb = const.tile([128, 2, NHT], F32, tag="bn1")
        nc.gpsimd.dma_start(out=bn1_sb, in_=bn1x)
        bn2_sb = const.tile([128, 2, NMT], F32, tag="bn2")
        nc.gpsimd.dma_start(out=bn2_sb, in_=bn2x)
        w1s_sb = const.tile([128, 3, H4], BF16, tag="w1s")
        nc.vector.memset(w1s_sb, 0.0)
        nc.gpsimd.dma_start(out=w1s_sb[:48], in_=w1s)
        doc_sb = const.tile([128, NKDOC, DD], BF16, tag="doc")
        nc.gpsimd.dma_start(out=doc_sb, in_=doc)
        w1_sb = const.tile([128, 6, H4], BF16, tag="w1")
        for c0, c1 in ((2, 4), (4, 6), (0, 2)):
            nc.gpsimd.dma_start(out=w1_sb[:, c0:c1, :], in_=w1[:, c0:c1, :])
        w3_sb = const.tile([128, NMT, 2], BF16, tag="w3")
        nc.gpsimd.dma_start(out=w3_sb, in_=w3)

        def w1_slice(u, kt, hs):
            # u in {l=0,c=1,r=2}; kt 0..2 (128/128/48 d-rows)
            if kt < 2:
                return w1_sb[:, u * 2 + kt, hs]
            return w1s_sb[:, u, hs]

        sc1 = bn1_sb[:, 0, :]     # scale1 per h-channel
        q1f = bn1_sb[:, 1, :]     # scale1*q1 + shift1 per h-channel (per batch)

        # ---- persistent G-domain tensors ----
        f_sb = const.tile([128, 3, WIN], BF16, tag="f_sb")
        r_sb = const.tile([128, 3, WIN], BF16, tag="r_sb")
        nc.vector.memset(f_sb, 0.0)
        nc.vector.memset(r_sb, 0.0)
        gc0_all = const.tile([128, NHT, NS], F16, tag="gc0")
        base_all = const.tile([128, NHT, NS], F16, tag="base")
        e_all = const.tile([128, NHT, EPAD], F16, tag="e_all")

        warmP = tc.alloc_tile_pool(name="warmP", bufs=1, space="PSUM")
        warm = warmP.tile([128, 512], F32, tag="warm")
        for i in range(NWARM_HEAD):
            nc.tensor.matmul(warm[:, 0:WIN], scratch[:, 0:128], scratch,
                             start=True, stop=True)

        with tc.tile_pool(name="psA", bufs=6, space="PSUM") as psA:
            # ---- F/R prefix GEMMs, kt-wave order: stalls on late amat
            # chunks stay short so the HAM clock-gate never re-throttles ----
            fr_ps = {}
            for dt, (d0, dsz) in enumerate(KT1):
                for half in range(2):
                    fr_ps[(dt, half)] = psA.tile([128, WIN], F32, tag="psA",
                                                 name=f"fr{dt}{half}")
            for kt in range(NKDOC):
                for dt, (d0, dsz) in enumerate(KT1):
                    for half in range(2):
                        nc.tensor.matmul(fr_ps[(dt, half)][:dsz],
                                         doc_sb[:, kt, d0:d0 + dsz],
                                         amat_sb[:, kt, half * WIN:(half + 1) * WIN],
                                         start=(kt == 0), stop=(kt == NKDOC - 1))
            for dt, (d0, dsz) in enumerate(KT1):
                for half, dst in ((0, f_sb), (1, r_sb)):
                    nc.scalar.activation(out=dst[:dsz, dt, :],
                                         in_=fr_ps[(dt, half)][:dsz],
                                         func=AF.Copy)

            # ---- G GEMMs; E = Gc[t] + Gr[t+1] accumulated in PSUM ----
            # gc0/E evictions on ScalarE (frees DVE; PE is gated on these),
            # base eviction on DVE (needs the two-scalar mult+add form).
            h1_of = {}
            z_of = {}

            def emit_asm_half(j, half, relu=True):
                hsl = slice(half * 4, half * 4 + 4)
                cjv = float(-(ALPHA ** (j + 1)))
                ag = work.tile([128, 4, NS], F16, tag=f"ag{half}",
                               name=f"ag{half}_{j}")
                nc.vector.tensor_scalar(out=ag, in0=gc0_all[:, hsl, :],
                                        scalar1=cjv, scalar2=None, op0=OP.mult)
                s1 = work.tile([128, 4, NS], F16, tag=f"s1{half}",
                               name=f"s1{half}_{j}")
                nc.vector.tensor_tensor(out=s1, in0=ag,
                                        in1=base_all[:, hsl, :], op=OP.add)
                z = work.tile([128, 4, NS], F16, tag=f"z{half}",
                              name=f"z{half}_{j}")
                nc.vector.tensor_tensor(out=z, in0=s1,
                                        in1=e_all[:, hsl, j + 1:j + 1 + NS],
                                        op=OP.add)
                z_of[(j, half)] = z
                if relu:
                    emit_relu_half(j, half)

            def emit_relu_half(j, half):
                z = z_of[(j, half)]
                if half == 0:
                    h1 = h1p.tile([128, 4, NS], BF16, tag="h1l",
                                  name=f"h1l_{j}")
                    nc.scalar.activation(out=h1, in_=z, func=AF.Relu,
                                         bias=0.0, scale=1.0)
                else:
                    h1 = h1p.tile([128, 4, 416], F8, tag="h1h",
                                  name=f"h1h_{j}")
                    nc.scalar.activation(out=h1[:, :, 0:NS], in_=z,
                                         func=AF.Relu, bias=0.0, scale=1.0)
                h1_of[(j, half)] = h1

            def emit_g_group(grp, on_act):
                hts = list(range(grp * 4, grp * 4 + 4))
                pse = {}
                for ht in hts:
                    hs = slice(ht * 128, (ht + 1) * 128)
                    ps = psA.tile([128, WIN], F32, tag="psA", name=f"pse{ht}")
                    pse[ht] = ps
                    for kt in range(3):
                        nc.tensor.matmul(ps, w1_slice(1, kt, hs),
                                         f_sb[:, kt, :], start=(kt == 0),
                                         stop=False, skip_group_check=True)
                    nc.scalar.activation(out=gc0_all[:, ht, :],
                                         in_=ps[:, 0:NS], func=AF.Copy,
                                         scale=sc1[:, ht:ht + 1])
                for ht in hts:
                    hs = slice(ht * 128, (ht + 1) * 128)
                    ps = pse[ht]
                    for kt in range(3):
                        nc.tensor.matmul(ps[:, 1:423], w1_slice(2, kt, hs),
                                         r_sb[:, kt, 2:424], start=False,
                                         stop=(kt == 2), skip_group_check=True)
                    nc.scalar.activation(out=e_all[:, ht, 0:WIN], in_=ps,
                                         func=AF.Copy,
                                         scale=sc1[:, ht:ht + 1])
                for ht in hts:
                    hs = slice(ht * 128, (ht + 1) * 128)
                    ps2 = psA.tile([128, NS], F32, tag="psA", name=f"gl{ht}")
                    for kt in range(3):
                        nc.tensor.matmul(ps2, w1_slice(0, kt, hs),
                                         f_sb[:, kt, 0:NS],
                                         start=(kt == 0), stop=(kt == 2))
                    nc.vector.tensor_scalar(out=base_all[:, ht, :], in0=ps2,
                                            scalar1=sc1[:, ht:ht + 1],
                                            scalar2=q1f[:, ht:ht + 1],
                                            op0=OP.mult, op1=OP.add)

            emit_g_group(0, on_act=True)
            emit_asm_half(0, 0, relu=False)   # lo-halves only need group-0
            emit_asm_half(1, 0, relu=False)   # outputs; DVE fills the g1 +
            emit_asm_half(2, 0, relu=False)   # warm2 window with them
            emit_asm_half(3, 0, relu=False)
            emit_g_group(1, on_act=False)
            emit_relu_half(0, 0)
            emit_relu_half(1, 0)
            emit_relu_half(2, 0)
            emit_relu_half(3, 0)

        # ---- mid warmup: bridge the PE gap while span-0 h1 is assembled ----
        for i in range(NWARM_MID):
            nc.tensor.matmul(warm[:, 0:WIN], scratch[:, 0:128], scratch,
                             start=True, stop=True)
        warmP.release()

        psB = ctx.enter_context(tc.tile_pool(name="psB", bufs=3, space="PSUM"))
        psl3 = ctx.enter_context(tc.tile_pool(name="psl3", bufs=2, space="PSUM"))

        def emit_asm(j):
            for half in range(2):
                if (j, half) not in h1_of:
                    emit_asm_half(j, half)

        def emit_l2l3(jb):
            h2_of = {}
            for mt in range(NMT):
                ms = slice(mt * 128, (mt + 1) * 128)
                psb = {}
                for pair in range(2):
                    psb[pair] = psB.tile([128, 2, 512], F32, tag="l2",
                                         name=f"l2_{jb}_{mt}_{pair}")
                # 2 bf16 : 1 fp8-DR interleave -- each 213ns DR weight
                # load hides under two 171ns bf16 matmuls. For jblk 0 the
                # pair-major order lets L2 start before spans 2-3 assemble.
                if jb == 0:
                    bs = [("B", kt, pair, jj2) for pair in range(2)
                          for kt in range(4) for jj2 in range(2)]
                    ds = [("D", ktp, pair, jj2) for pair in range(2)
                          for ktp in range(2) for jj2 in range(2)]
                    seq = bs + ds
                else:
                    bs = [("B", kt, pair, jj2) for kt in range(4)
                          for pair in range(2) for jj2 in range(2)]
                    ds = [("D", ktp, pair, jj2) for ktp in range(2)
                          for pair in range(2) for jj2 in range(2)]
                    seq = []
                    for q in range(8):
                        seq += [bs[2 * q], bs[2 * q + 1], ds[q]]
                first, last = {}, {}
                for n_, it in enumerate(seq):
                    key = (it[2], it[3])
                    if key not in first:
                        first[key] = n_
                    last[key] = n_
                for n_, (typ, k_, pair, jj2) in enumerate(seq):
                    j = jb * 4 + pair * 2 + jj2
                    key = (pair, jj2)
                    if typ == "B":
                        nc.tensor.matmul(psb[pair][:, jj2, 0:NS],
                                         w2_sb[:, k_, ms],
                                         h1_of[(j, 0)][:, k_, :],
                                         start=(first[key] == n_),
                                         stop=(last[key] == n_),
                                         skip_group_check=True)
                    else:
                        nc.tensor.matmul(psb[pair][:, jj2, 0:NS],
                                         w2f8_sb[:, k_, :, ms],
                                         h1_of[(j, 1)][:, 2 * k_:2 * k_ + 2, 0:NS],
                                         start=(first[key] == n_),
                                         stop=(last[key] == n_),
                                         perf_mode=DR,
                                         skip_group_check=True)
                for pair in range(2):
                    h2 = h2p.tile([128, 2, NS], BF16, tag="h2",
                                  name=f"h2_{jb}_{mt}_{pair}")
                    nc.scalar.activation(out=h2, in_=psb[pair][:, :, 0:NS],
                                         func=AF.Relu,
                                         bias=bn2_sb[:, 1, mt:mt + 1],
                                         scale=bn2_sb[:, 0, mt:mt + 1])
                    h2_of[(mt, pair)] = h2
            ybig = yp.tile([2, 4, NS], F32, tag="y", name=f"y_{jb}")
            for jj in range(4):
                j = jb * 4 + jj
                pair, jj2 = jj // 2, jj % 2
                ps3 = psl3.tile([2, 512], F32, tag="l3", name=f"l3_{jb}_{jj}")
                for mt in range(NMT):
                    nc.tensor.matmul(ps3[:, 0:NS], w3_sb[:, mt, :],
                                     h2_of[(mt, pair)][:, jj2, :],
                                     start=(mt == 0), stop=(mt == NMT - 1))
                nc.vector.tensor_copy(ybig[:, jj, :], ps3[:, 0:NS])
            nc.sync.dma_start(out=y[:, jb * 4:jb * 4 + 4, :], in_=ybig)

        def emit_l2l3_tail(jb):
            ybig = yp.tile([2, 4, NS], F32, tag="y", name=f"y_{jb}")
            h2s = {}

            def l2_mt(pair, mt):
                ms = slice(mt * 128, (mt + 1) * 128)
                psb = psB.tile([128, 2, 512], F32, tag="l2",
                               name=f"l2_{jb}_{mt}_{pair}")
                bs = [("B", kt, jj2) for kt in range(4) for jj2 in range(2)]
                ds = [("D", ktp, jj2) for ktp in range(2) for jj2 in range(2)]
                seq = []
                for q in range(4):
                    seq += [bs[2 * q], bs[2 * q + 1], ds[q]]
                first, last = {}, {}
                for n_, it in enumerate(seq):
                    if it[2] not in first:
                        first[it[2]] = n_
                    last[it[2]] = n_
                for n_, (typ, k_, jj2) in enumerate(seq):
                    j = jb * 4 + pair * 2 + jj2
                    if typ == "B":
                        nc.tensor.matmul(psb[:, jj2, 0:NS], w2_sb[:, k_, ms],
                                         h1_of[(j, 0)][:, k_, :],
                                         start=(first[jj2] == n_),
                                         stop=(last[jj2] == n_),
                                         skip_group_check=True)
                    else:
                        nc.tensor.matmul(psb[:, jj2, 0:NS],
                                         w2f8_sb[:, k_, :, ms],
                                         h1_of[(j, 1)][:, 2 * k_:2 * k_ + 2, 0:NS],
                                         start=(first[jj2] == n_),
                                         stop=(last[jj2] == n_),
                                         perf_mode=DR, skip_group_check=True)
                h2 = h2p.tile([128, 2, NS], BF16, tag="h2",
                              name=f"h2_{jb}_{mt}_{pair}")
                nc.scalar.activation(out=h2, in_=psb[:, :, 0:NS], func=AF.Relu,
                                     bias=bn2_sb[:, 1, mt:mt + 1],
                                     scale=bn2_sb[:, 0, mt:mt + 1])
                h2s[(pair, mt)] = h2

            def l3_jj(pair, jj2):
                jj = pair * 2 + jj2
                ps3 = psl3.tile([2, 512], F32, tag="l3", name=f"l3_{jb}_{jj}")
                for mt in range(NMT):
                    nc.tensor.matmul(ps3[:, 0:NS], w3_sb[:, mt, :],
                                     h2s[(pair, mt)][:, jj2, :],
                                     start=(mt == 0), stop=(mt == NMT - 1))
                nc.vector.tensor_copy(ybig[:, jj, :], ps3[:, 0:NS])

            for mt in range(NMT):
                l2_mt(0, mt)
            l2_mt(1, 0)
            l2_mt(1, 1)
            l3_jj(0, 0)
            l2_mt(1, 2)
            l3_jj(0, 1)
            nc.sync.dma_start(out=y[:, jb * 4:jb * 4 + 2, :],
                              in_=ybig[:, 0:2, :])
            l2_mt(1, 3)
            l3_jj(1, 0)
            l3_jj(1, 1)
            nc.sync.dma_start(out=y[:, jb * 4 + 2:jb * 4 + 4, :],
                              in_=ybig[:, 2:4, :])

        # software-pipelined emission: assembly runs 1-2 jblks ahead of L2/L3
        for j in range(0, 8):
            emit_asm(j)
        emit_l2l3(0)
        for j in range(8, 12):
            emit_asm(j)
        emit_l2l3(1)
        for j in range(12, 16):
            emit_asm(j)
        emit_l2l3(2)
        emit_l2l3_tail(3)

    nc.compile()
    return nc


def _get_nc():
    if "nc" not in _CACHE:
        _CACHE["nc"] = _build_bass()
    return _CACHE["nc"]


def _ensure_device():
    """Probe the axon device; reset it if wedged."""
    if _CACHE.get("dev_ok"):
        return
    import jax
    import jax.numpy as jnp
    try:
        (jnp.zeros((8, 8)) + 1).block_until_ready()
    except Exception:
        import ctypes
        lib = ctypes.CDLL("/opt/axon/libaxon_pjrt.so")
        lib.axon_reset.restype = ctypes.c_int64
        jax.devices()
        lib.axon_reset()
        (jnp.zeros((8, 8)) + 1).block_until_ready()
    _CACHE["dev_ok"] = True


def _chan_split(doc_emb, W1, scale1, shift1, W2, q1max):
    """Rank h-channels by variance contribution to z2; light 512 go fp8.

    Returns (perm, s) with s the per-channel fp8 fold scale (1 for heavy).
    """
    A = ALPHA ** np.maximum(np.subtract.outer(np.arange(T), np.arange(T)), 0)
    A = np.where(np.subtract.outer(np.arange(T), np.arange(T)) >= 0, A, 0.0)
    A = A.astype(np.float32)
    U_l, U_c, U_r = W1[:, :DD], W1[:, DD:2 * DD], W1[:, 2 * DD:3 * DD]
    F = np.einsum('tk,bkd->btd', A, doc_emb)
    R = np.einsum('tk,bkd->btd', A.T, doc_emb)
    Gc = F @ U_c.T
    Gl = np.concatenate([np.zeros((B, 1, H4), np.float32),
                         F @ U_l.T], axis=1)[:, :T]
    Rp = np.concatenate([R, np.zeros((B, 1, DD), np.float32)], axis=1)
    Gr = Rp[:, 1:T + 1] @ U_r.T
    s_idx, e_idx = _CACHE.setdefault("cands", _cand_indices())
    sub = slice(0, None, 16)
    ss, ee = s_idx[sub], e_idx[sub]
    coef = (ALPHA ** (ee - ss + 1)).astype(np.float32)[None, :, None]
    Gcp = np.concatenate([np.zeros((B, 1, H4), np.float32), Gc], axis=1)
    z1s = Gl[:, ss] + Gc[:, ee] - coef * Gcp[:, ss] + Gr[:, ee]
    z1s = scale1 * z1s + shift1          # q1 term omitted: minor for ranking
    h2m = np.maximum(z1s, 0) ** 2
    contrib = h2m.mean(axis=(0, 1)) * (W2 ** 2).mean(axis=0)
    order = np.argsort(contrib)
    perm = np.concatenate([np.sort(order[512:]), np.sort(order[:512])])
    light = perm[512:]
    gmax = (np.abs(Gl).max(axis=(0, 1)) + (1 + ALPHA) * np.abs(Gc).max(axis=(0, 1))
            + np.abs(Gr).max(axis=(0, 1)))
    bound = np.abs(scale1) * (gmax + q1max) + np.abs(shift1) + 1e-6
    s = np.ones(H4, np.float32)
    s[light] = np.minimum(1.0, 240.0 / (1.1 * bound[light]))
    return perm, s


def _make_in_maps(inputs):
    import ml_dtypes
    doc_emb = np.asarray(inputs["doc_emb"], np.float32)
    query_emb = np.asarray(inputs["query_emb"], np.float32)
    W1 = np.asarray(inputs["W1"], np.float32)
    W2 = np.asarray(inputs["W2"], np.float32)
    W3 = np.asarray(inputs["W3"], np.float32)
    g1, b1, m1, v1 = (np.asarray(inputs[k], np.float32) for k in ("g1", "b1", "m1", "v1"))
    g2, b2, m2, v2 = (np.asarray(inputs[k], np.float32) for k in ("g2", "b2", "m2", "v2"))

    scale1 = g1 / np.sqrt(v1 + BN_EPS)
    shift1 = b1 - m1 * scale1
    scale2 = g2 / np.sqrt(v2 + BN_EPS)
    shift2 = b2 - m2 * scale2

    # channel permutation: heavy 512 first (bf16), light 512 last (fp8e4-DR);
    # fp8 overflow-guard scale s folded into bn1 stats and W2 columns.
    wv0 = ALPHA ** np.arange(LQ - 1, -1, -1, dtype=np.float32)
    q1max = np.abs(np.einsum('t,btd->bd', wv0, query_emb)
                   @ W1[:, 3 * DD:].T).max(axis=0)
    perm, s = _chan_split(doc_emb, W1, scale1, shift1, W2, q1max)
    W1 = W1[perm]
    W2 = W2[:, perm]
    scale1 = scale1[perm] * s[perm]
    shift1 = shift1[perm] * s[perm]
    W2 = W2 / s[perm][None, :]

    # host q-path: qf[b] = sum_t a^(Lq-1-t) q[b,t];  q1f[b] = scale1*(U_q qf) + shift1
    wv = ALPHA ** np.arange(LQ - 1, -1, -1, dtype=np.float32)
    qf = np.einsum('t,btd->bd', wv, query_emb)
    q1 = qf @ W1[:, 3 * DD:].T                     # [B, H4]
    q1f = scale1[None, :] * q1 + shift1[None, :]   # [B, H4]

    W1T = np.ascontiguousarray(W1.T)               # [1212, 1024]
    w1_h = np.zeros((6, 128, H4), ml_dtypes.bfloat16)
    w1s_h = np.zeros((3, 48, H4), ml_dtypes.bfloat16)
    for u in range(3):
        for kt, (k0, ksz) in enumerate(KT1):
            blk = W1T[u * DD + k0: u * DD + k0 + ksz]
            if kt < 2:
                w1_h[u * 2 + kt] = blk
            else:
                w1s_h[u] = blk
    w1_h = np.ascontiguousarray(w1_h.transpose(1, 0, 2))
    w1s_h = np.ascontiguousarray(w1s_h.transpose(1, 0, 2))

    W2T = W2.T                                     # [1024, 512], perm'd+scaled
    w2_h = np.ascontiguousarray(
        W2T[:512].reshape(4, 128, H2).transpose(1, 0, 2)).astype(ml_dtypes.bfloat16)
    w2f8_h = np.ascontiguousarray(
        np.clip(W2T[512:], -240, 240).reshape(2, 2, 128, H2)
        .transpose(2, 0, 1, 3)).astype(ml_dtypes.float8_e4m3)
    w3_h = np.ascontiguousarray(
        W3.T.reshape(NMT, 128, 2).transpose(1, 0, 2)).astype(ml_dtypes.bfloat16)

    bn2x = np.ascontiguousarray(
        np.stack([scale2.reshape(NMT, 128).T, shift2.reshape(NMT, 128).T],
                 axis=1))                           # [128, 2, NMT]

    amats = [_pack5(_build_amat(0), 2 * WIN, 0).astype(ml_dtypes.bfloat16),
             _pack5(_build_amat(403), 2 * WIN, 169).astype(ml_dtypes.bfloat16)]

    in_maps = []
    for core in range(N_CORES):
        b, half = core // 2, core % 2
        bn1x = np.ascontiguousarray(
            np.stack([scale1.reshape(NHT, 128).T,
                      q1f[b].reshape(NHT, 128).T], axis=1))   # [128, 2, NHT]
        in_maps.append({
            "doc": _pack5(doc_emb[b], DD, 169 * half).astype(ml_dtypes.bfloat16),
            "amat": amats[half],
            "w1": w1_h,
            "w1s": w1s_h,
            "w2": w2_h,
            "w2f8": w2f8_h,
            "w3": w3_h,
            "bn1x": bn1x,
            "bn2x": bn2x,
        })
    return in_maps


def _gather(results):
    s_idx, e_idx = _CACHE.setdefault("cands", _cand_indices())
    n = len(s_idx)
    j_idx = e_idx - s_idx
    half_idx = (s_idx >= 406).astype(np.int64)
    u_idx = s_idx - 403 * half_idx
    out = np.zeros((B, n, 2), np.float32)
    for b in range(B):
        both = np.stack([results[2 * b]["y"], results[2 * b + 1]["y"]])  # [2,2,16,NS]
        out[b] = both[half_idx, :, j_idx, u_idx]
    return out


def _run(inputs, trace=False):
    from concourse import bass_utils
    _ensure_device()
    nc = _get_nc()
    in_maps = _make_in_maps(inputs)
    res = bass_utils.run_bass_kernel_spmd(nc, in_maps,
                                          core_ids=list(range(N_CORES)),
                                          trace=trace)
    return _gather(res.results), res


def kernel(**inputs) -> np.ndarray:
    out, _ = _run(inputs, trace=False)
    return out


# revision 33
# speedup vs baseline: 1.1944x; 1.1944x over previous
"""FOFEReader Trainium2 kernel: 8-core SPMD (batch x s-half sharding), v2.

Math (per batch b, candidate (s, e=s+j), j<16):
  F[t] = sum_{k<=t} a^(t-k) doc[k]   (prefix FOFE),  R[t] = sum_{k>=t} a^(k-t) doc[k]
  x = [F[s-1] | F[s+j] - a^(j+1) F[s-1] | R[s+j+1] | qf]
  out = (relu(bn2(relu(bn1(x @ W1.T)) @ W2.T)) @ W3.T)
Reformulated so the 1212-dim GEMM is shared across the 16 spans j:
  G_u = U_u @ F (u in {l,c}), G_r = U_r @ R   with W1.T = [U_l U_c U_r U_q] row blocks
  z[s,j] = (G_l[s-1] + q1)' + (G_c[s+j] + G_r[s+j+1])' - a^(j+1) G_c[s-1]'
where ' marks the bn1 scale folded in at PSUM eviction.

v2 structure (vs v1):
  - bf16 for the L2/L3 matmul operands (fp16 moving runs ~1.2x slower on PE);
    z assembly stays fp16 for mantissa.
  - E = Gc[t] + Gr[t+1] accumulated IN PSUM via a shifted matmul output AP
    (one eviction instead of two + a vector add).
  - q-path computed on host, shipped via bn1x (kills ~30 device instructions).
  - batched DVE assembly: one TS (4x mode) + two TT (2x mode) over [128,8,406]
    per span instead of 16 per-tile ops; one ScalarE relu per span.
  - L2 runs 4 spans per weight load (jblk=4) into 2-bank PSUM tiles; h2
    eviction is one activation per (mt, span-pair).
  - warmup matmul bursts keep the PE HAM clock-gate at 8/8 from ~4us.
  - single straight DMA per input tensor (host pre-packs SBUF layouts).
"""
import os
import sys

for _p in ("/opt/trn_rl_repo", "/root/.axon_site/_ro/trn_rl_repo"):
    if os.path.isdir(_p) and _p not in sys.path:
        sys.path.insert(0, _p)
        break

import numpy as np

T = 809
MSPAN = 16
B = 4
ALPHA = 0.9
NS = 406          # s-starts per core
WIN = 424         # t window per core: t = s_lo-1 + i, i in [0, 424)
DD = 304
EMB = 300
LQ = 30
H4 = 1024
H2 = 512
BN_EPS = 1e-5
N_CORES = 8
NHT = H4 // 128   # 8
NMT = H2 // 128   # 4
NWARM_HEAD = 44
NWARM_MID = 30
EPAD = 432        # padded E row length (keeps slices 2-byte packed)

_CACHE = {}

KT1 = [(0, 128), (128, 128), (256, 48)]       # d-tiles of 304
NKDOC = 5                                     # 640-row per-core token window


def _build_amat(s_lo):
    """[809, 848] fp32: cols 0..423 = forward-FOFE operator columns for
    t=s_lo-1+i (A^T slice), cols 424.. = reverse. Out-of-range t -> zero col."""
    t_idx = s_lo - 1 + np.arange(WIN)
    kv = np.arange(T)[:, None]
    tv = t_idx[None, :]
    valid = ((t_idx >= 0) & (t_idx <= T - 1))[None, :]
    af = np.where((kv <= tv) & valid, ALPHA ** np.maximum(tv - kv, 0), 0.0)
    ar = np.where((kv >= tv) & valid, ALPHA ** np.maximum(kv - tv, 0), 0.0)
    return np.concatenate([af, ar], axis=1).astype(np.float32)


def _cand_indices():
    s_list, e_list = [], []
    for s in range(T):
        for span in range(min(MSPAN, T - s)):
            s_list.append(s)
            e_list.append(s + span)
    return np.asarray(s_list, np.int64), np.asarray(e_list, np.int64)


def _pack5(a, width, w0):
    """rows [w0, w0+640) of [809, width] -> [128, 5, width] partition-major.
    FOFE decay makes rows outside the window contribute < 1e-9 relative."""
    out = np.ascontiguousarray(a[w0:w0 + 640])
    return np.ascontiguousarray(out.reshape(5, 128, width).transpose(1, 0, 2))


def _build_bass():
    import concourse.bacc as bacc
    import concourse.tile as tile
    from concourse import mybir
    from contextlib import ExitStack

    F32 = mybir.dt.float32
    F16 = mybir.dt.float16
    BF16 = mybir.dt.bfloat16
    F8 = mybir.dt.float8e4
    DR = mybir.MatmulPerfMode.DoubleRow
    AF = mybir.ActivationFunctionType
    OP = mybir.AluOpType

    nc = bacc.Bacc("TRN2", target_bir_lowering=False, debug=False,
                   num_devices=N_CORES)

    doc = nc.dram_tensor("doc", [128, NKDOC, DD], BF16, kind="ExternalInput").ap()
    amat = nc.dram_tensor("amat", [128, NKDOC, 2 * WIN], BF16, kind="ExternalInput").ap()
    w1 = nc.dram_tensor("w1", [128, 6, H4], BF16, kind="ExternalInput").ap()
    w1s = nc.dram_tensor("w1s", [48, 3, H4], BF16, kind="ExternalInput").ap()
    w2 = nc.dram_tensor("w2", [128, 4, H2], BF16, kind="ExternalInput").ap()
    w2f8 = nc.dram_tensor("w2f8", [128, 2, 2, H2], F8, kind="ExternalInput").ap()
    w3 = nc.dram_tensor("w3", [128, NMT, 2], BF16, kind="ExternalInput").ap()
    bn1x = nc.dram_tensor("bn1x", [128, 2, NHT], F32, kind="ExternalInput").ap()
    bn2x = nc.dram_tensor("bn2x", [128, 2, NMT], F32, kind="ExternalInput").ap()
    y = nc.dram_tensor("y", [2, MSPAN, NS], F32, kind="ExternalOutput").ap()

    with ExitStack() as ctx:
        tc = ctx.enter_context(tile.TileContext(nc))
        const = ctx.enter_context(tc.tile_pool(name="const", bufs=1))
        work = ctx.enter_context(tc.tile_pool(name="work", bufs=2))
        h1p = ctx.enter_context(tc.tile_pool(name="h1p", bufs=8))
        h2p = ctx.enter_context(tc.tile_pool(name="h2p", bufs=6))
        yp = ctx.enter_context(tc.tile_pool(name="yp", bufs=3))

        # ---- scratch for warmup (no DMA dependency) ----
        scratch = const.tile([128, WIN], BF16, tag="scratch")
        nc.vector.memset(scratch, 0.01)

        # ---- chunked DMAs: transfers start early and pipeline with use ----
        # sync queue: amat (chunks, F/R critical path) then w2
        # gpsimd queue: bn, doc, w1 (chunks), w3
        amat_sb = const.tile([128, NKDOC, 2 * WIN], BF16, tag="amat")
        for c0, c1 in ((0, 2), (2, 4), (4, NKDOC)):
            nc.sync.dma_start(out=amat_sb[:, c0:c1, :], in_=amat[:, c0:c1, :])
        w2_sb = const.tile([128, 4, H2], BF16, tag="w2")
        for c0, c1 in ((0, 2), (2, 4)):
            nc.sync.dma_start(out=w2_sb[:, c0:c1, :], in_=w2[:, c0:c1, :])
        w2f8_sb = const.tile([128, 2, 2, H2], F8, tag="w2f8")
        nc.sync.dma_start(out=w2f8_sb, in_=w2f8)
        bn1_sb = const.tile([128, 2, NHT], F32, tag="bn1")
        nc.gpsimd.dma_start(out=bn1_sb, in_=bn1x)
        bn2_sb = const.tile([128, 2, NMT], F32, tag="bn2")
        nc.gpsimd.dma_start(out=bn2_sb, in_=bn2x)
        w1s_sb = const.tile([128, 3, H4], BF16, tag="w1s")
        nc.vector.memset(w1s_sb, 0.0)
        nc.gpsimd.dma_start(out=w1s_sb[:48], in_=w1s)
        doc_sb = const.tile([128, NKDOC, DD], BF16, tag="doc")
        nc.gpsimd.dma_start(out=doc_sb, in_=doc)
        w1_sb = const.tile([128, 6, H4], BF16, tag="w1")
        for c0, c1 in ((2, 4), (4, 6), (0, 2)):
            nc.gpsimd.dma_start(out=w1_sb[:, c0:c1, :], in_=w1[:, c0:c1, :])
        w3_sb = const.tile([128, NMT, 2], BF16, tag="w3")
        nc.gpsimd.dma_start(out=w3_sb, in_=w3)

        def w1_slice(u, kt, hs):
            # u in {l=0,c=1,r=2}; kt 0..2 (128/128/48 d-rows)
            if kt < 2:
                return w1_sb[:, u * 2 + kt, hs]
            return w1s_sb[:, u, hs]

        sc1 = bn1_sb[:, 0, :]     # scale1 per h-channel
        q1f = bn1_sb[:, 1, :]     # scale1*q1 + shift1 per h-channel (per batch)

        # ---- persistent G-domain tensors ----
        f_sb = const.tile([128, 3, WIN], BF16, tag="f_sb")
        r_sb = const.tile([128, 3, WIN], BF16, tag="r_sb")
        nc.vector.memset(f_sb, 0.0)
        nc.vector.memset(r_sb, 0.0)
        gc0_all = const.tile([128, NHT, NS], F16, tag="gc0")
        base_all = const.tile([128, NHT, NS], F16, tag="base")
        e_all = const.tile([128, NHT, EPAD], F16, tag="e_all")

        warmP = tc.alloc_tile_pool(name="warmP", bufs=1, space="PSUM")
        warm = warmP.tile([128, 512], F32, tag="warm")
        for i in range(NWARM_HEAD):
            nc.tensor.matmul(warm[:, 0:WIN], scratch[:, 0:128], scratch,
                             start=True, stop=True)

        with tc.tile_pool(name="psA", bufs=6, space="PSUM") as psA:
            # ---- F/R prefix GEMMs, kt-wave order: stalls on late amat
            # chunks stay short so the HAM clock-gate never re-throttles ----
            fr_ps = {}
            for dt, (d0, dsz) in enumerate(KT1):
                for half in range(2):
                    fr_ps[(dt, half)] = psA.tile([128, WIN], F32, tag="psA",
                                                 name=f"fr{dt}{half}")
            for kt in range(NKDOC):
                for dt, (d0, dsz) in enumerate(KT1):
                    for half in range(2):
                        nc.tensor.matmul(fr_ps[(dt, half)][:dsz],
                                         doc_sb[:, kt, d0:d0 + dsz],
                                         amat_sb[:, kt, half * WIN:(half + 1) * WIN],
                                         start=(kt == 0), stop=(kt == NKDOC - 1))
            for dt, (d0, dsz) in enumerate(KT1):
                for half, dst in ((0, f_sb), (1, r_sb)):
                    nc.scalar.activation(out=dst[:dsz, dt, :],
                                         in_=fr_ps[(dt, half)][:dsz],
                                         func=AF.Copy)

            # ---- G GEMMs; E = Gc[t] + Gr[t+1] accumulated in PSUM ----
            # gc0/E evictions on ScalarE (frees DVE; PE is gated on these),
            # base eviction on DVE (needs the two-scalar mult+add form).
            h1_of = {}
            z_of = {}

            def emit_asm_half(j, half, relu=True):
                hsl = slice(half * 4, half * 4 + 4)
                cjv = float(-(ALPHA ** (j + 1)))
                ag = work.tile([128, 4, NS], F16, tag=f"ag{half}",
                               name=f"ag{half}_{j}")
                nc.vector.tensor_scalar(out=ag, in0=gc0_all[:, hsl, :],
                                        scalar1=cjv, scalar2=None, op0=OP.mult)
                s1 = work.tile([128, 4, NS], F16, tag=f"s1{half}",
                               name=f"s1{half}_{j}")
                nc.vector.tensor_tensor(out=s1, in0=ag,
                                        in1=base_all[:, hsl, :], op=OP.add)
                z = work.tile([128, 4, NS], F16, tag=f"z{half}",
                              name=f"z{half}_{j}")
                nc.vector.tensor_tensor(out=z, in0=s1,
                                        in1=e_all[:, hsl, j + 1:j + 1 + NS],
                                        op=OP.add)
                z_of[(j, half)] = z
                if relu:
                    emit_relu_half(j, half)

            def emit_relu_half(j, half):
                z = z_of[(j, half)]
                if half == 0:
                    h1 = h1p.tile([128, 4, NS], BF16, tag="h1l",
                                  name=f"h1l_{j}")
                    nc.scalar.activation(out=h1, in_=z, func=AF.Relu,
                                         bias=0.0, scale=1.0)
                else:
                    h1 = h1p.tile([128, 4, 416], F8, tag="h1h",
                                  name=f"h1h_{j}")
                    nc.scalar.activation(out=h1[:, :, 0:NS], in_=z,
                                         func=AF.Relu, bias=0.0, scale=1.0)
                h1_of[(j, half)] = h1

            def emit_g_group(grp, on_act):
                hts = list(range(grp * 4, grp * 4 + 4))
                pse = {}
                for ht in hts:
                    hs = slice(ht * 128, (ht + 1) * 128)
                    ps = psA.tile([128, WIN], F32, tag="psA", name=f"pse{ht}")
                    pse[ht] = ps
                    for kt in range(3):
                        nc.tensor.matmul(ps, w1_slice(1, kt, hs),
                                         f_sb[:, kt, :], start=(kt == 0),
                                         stop=False, skip_group_check=True)
                    nc.scalar.activation(out=gc0_all[:, ht, :],
                                         in_=ps[:, 0:NS], func=AF.Copy,
                                         scale=sc1[:, ht:ht + 1])
                for ht in hts:
                    hs = slice(ht * 128, (ht + 1) * 128)
                    ps = pse[ht]
                    for kt in range(3):
                        nc.tensor.matmul(ps[:, 1:423], w1_slice(2, kt, hs),
                                         r_sb[:, kt, 2:424], start=False,
                                         stop=(kt == 2), skip_group_check=True)
                    nc.scalar.activation(out=e_all[:, ht, 0:WIN], in_=ps,
                                         func=AF.Copy,
                                         scale=sc1[:, ht:ht + 1])
                for ht in hts:
                    hs = slice(ht * 128, (ht + 1) * 128)
                    ps2 = psA.tile([128, NS], F32, tag="psA", name=f"gl{ht}")
                    for kt in range(3):
                        nc.tensor.matmul(ps2, w1_slice(0, kt, hs),
                                         f_sb[:, kt, 0:NS],
                                         start=(kt == 0), stop=(kt == 2))
                    nc.vector.tensor_scalar(out=base_all[:, ht, :], in0=ps2,
                                            scalar1=sc1[:, ht:ht + 1],
                                            scalar2=q1f[:, ht:ht + 1],
                                            op0=OP.mult, op1=OP.add)

            emit_g_group(0, on_act=True)
            emit_asm_half(0, 0, relu=False)   # lo-halves only need group-0
            emit_asm_half(1, 0, relu=False)   # outputs; DVE fills the g1 +
            emit_asm_half(2, 0, relu=False)   # warm2 window with them
            emit_asm_half(3, 0, relu=False)
            emit_g_group(1, on_act=False)
            emit_relu_half(0, 0)
            emit_relu_half(1, 0)
            emit_relu_half(2, 0)
            emit_relu_half(3, 0)

        # ---- mid warmup: bridge the PE gap while span-0 h1 is assembled ----
        for i in range(NWARM_MID):
            nc.tensor.matmul(warm[:, 0:WIN], scratch[:, 0:128], scratch,
                             start=True, stop=True)
        warmP.release()

        psB = ctx.enter_context(tc.tile_pool(name="psB", bufs=3, space="PSUM"))
        psl3 = ctx.enter_context(tc.tile_pool(name="psl3", bufs=2, space="PSUM"))

        def emit_asm(j):
            for half in range(2):
                if (j, half) not in h1_of:
                    emit_asm_half(j, half)

        def emit_l2l3(jb):
            h2_of = {}
            for mt in range(NMT):
                ms = slice(mt * 128, (mt + 1) * 128)
                psb = {}
                for pair in range(2):
                    psb[pair] = psB.tile([128, 2, 512], F32, tag="l2",
                                         name=f"l2_{jb}_{mt}_{pair}")
                # 2 bf16 : 1 fp8-DR interleave -- each 213ns DR weight
                # load hides under two 171ns bf16 matmuls. For jblk 0 the
                # pair-major order lets L2 start before spans 2-3 assemble.
                if jb == 0:
                    bs = [("B", kt, pair, jj2) for pair in range(2)
                          for kt in range(4) for jj2 in range(2)]
                    ds = [("D", ktp, pair, jj2) for pair in range(2)
                          for ktp in range(2) for jj2 in range(2)]
                    seq = bs + ds
                else:
                    bs = [("B", kt, pair, jj2) for kt in range(4)
                          for pair in range(2) for jj2 in range(2)]
                    ds = [("D", ktp, pair, jj2) for ktp in range(2)
                          for pair in range(2) for jj2 in range(2)]
                    seq = []
                    for q in range(8):
                        seq += [bs[2 * q], bs[2 * q + 1], ds[q]]
                first, last = {}, {}
                for n_, it in enumerate(seq):
                    key = (it[2], it[3])
                    if key not in first:
                        first[key] = n_
                    last[key] = n_
                for n_, (typ, k_, pair, jj2) in enumerate(seq):
                    j = jb * 4 + pair * 2 + jj2
                    key = (pair, jj2)
                    if typ == "B":
                        nc.tensor.matmul(psb[pair][:, jj2, 0:NS],
                                         w2_sb[:, k_, ms],
                                         h1_of[(j, 0)][:, k_, :],
                                         start=(first[key] == n_),
                                         stop=(last[key] == n_),
                                         skip_group_check=True)
                    else:
                        nc.tensor.matmul(psb[pair][:, jj2, 0:NS],
                                         w2f8_sb[:, k_, :, ms],
                                         h1_of[(j, 1)][:, 2 * k_:2 * k_ + 2, 0:NS],
                                         start=(first[key] == n_),
                                         stop=(last[key] == n_),
                                         perf_mode=DR,
                                         skip_group_check=True)
                for pair in range(2):
                    h2 = h2p.tile([128, 2, NS], BF16, tag="h2",
                                  name=f"h2_{jb}_{mt}_{pair}")
                    nc.scalar.activation(out=h2, in_=psb[pair][:, :, 0:NS],
                                         func=AF.Relu,
                                         bias=bn2_sb[:, 1, mt:mt + 1],
                                         scale=bn2_sb[:, 0, mt:mt + 1])
                    h2_of[(mt, pair)] = h2
            ybig = yp.tile([2, 4, NS], F32, tag="y", name=f"y_{jb}")
            for jj in range(4):
                j = jb * 4 + jj
                pair, jj2 = jj // 2, jj % 2
                ps3 = psl3.tile([2, 512], F32, tag="l3", name=f"l3_{jb}_{jj}")
                for mt in range(NMT):
                    nc.tensor.matmul(ps3[:, 0:NS], w3_sb[:, mt, :],
                                     h2_of[(mt, pair)][:, jj2, :],
                                     start=(mt == 0), stop=(mt == NMT - 1))
                nc.vector.tensor_copy(ybig[:, jj, :], ps3[:, 0:NS])
            nc.sync.dma_start(out=y[:, jb * 4:jb * 4 + 4, :], in_=ybig)

        def emit_l2l3_tail(jb):
            ybig = yp.tile([2, 4, NS], F32, tag="y", name=f"y_{jb}")
            h2s = {}

            def l2_mt(pair, mt):
                ms = slice(mt * 128, (mt + 1) * 128)
                psb = psB.tile([128, 2, 512], F32, tag="l2",
                               name=f"l2_{jb}_{mt}_{pair}")
                bs = [("B", kt, jj2) for kt in range(4) for jj2 in range(2)]
                ds = [("D", ktp, jj2) for ktp in range(2) for jj2 in range(2)]
                seq = []
                for q in range(4):
                    seq += [bs[2 * q], bs[2 * q + 1], ds[q]]
                first, last = {}, {}
                for n_, it in enumerate(seq):
                    if it[2] not in first:
                        first[it[2]] = n_
                    last[it[2]] = n_
                for n_, (typ, k_, jj2) in enumerate(seq):
                    j = jb * 4 + pair * 2 + jj2
                    if typ == "B":
                        nc.tensor.matmul(psb[:, jj2, 0:NS], w2_sb[:, k_, ms],
                                         h1_of[(j, 0)][:, k_, :],
                                         start=(first[jj2] == n_),
                                         stop=(last[jj2] == n_),
                                         skip_group_check=True)
                    else:
                        nc.tensor.matmul(psb[:, jj2, 0:NS],
                                         w2f8_sb[:, k_, :, ms],
                                         h1_of[(j, 1)][:, 2 * k_:2 * k_ + 2, 0:NS],
                                         start=(first[jj2] == n_),
                                         stop=(last[jj2] == n_),
                                         perf_mode=DR, skip_group_check=True)
                h2 = h2p.tile([128, 2, NS], BF16, tag="h2",
                              name=f"h2_{jb}_{mt}_{pair}")
                nc.scalar.activation(out=h2, in_=psb[:, :, 0:NS], func=AF.Relu,
                                     bias=bn2_sb[:, 1, mt:mt + 1],
                                     scale=bn2_sb[:, 0, mt:mt + 1])
                h2s[(pair, mt)] = h2

            def l3_jj(pair, jj2):
                jj = pair * 2 + jj2
                ps3 = psl3.tile([2, 512], F32, tag="l3", name=f"l3_{jb}_{jj}")
                for mt in range(NMT):
                    nc.tensor.matmul(ps3[:, 0:NS], w3_sb[:, mt, :],
                                     h2s[(pair, mt)][:, jj2, :],
                                     start=(mt == 0), stop=(mt == NMT - 1))
                nc.vector.tensor_copy(ybig[:, jj, :], ps3[:, 0:NS])

            for mt in range(NMT):
                l2_mt(0, mt)
            l2_mt(1, 0)
            l2_mt(1, 1)
            l3_jj(0, 0)
            l2_mt(1, 2)
            l3_jj(0, 1)
            nc.sync.dma_start(out=y[:, jb * 4:jb * 4 + 2, :],
                              in_=ybig[:, 0:2, :])
            l2_mt(1, 3)
            l3_jj(1, 0)
            l3_jj(1, 1)
            nc.sync.dma_start(out=y[:, jb * 4 + 2:jb * 4 + 4, :],
                              in_=ybig[:, 2:4, :])

        # software-pipelined emission: assembly runs 1-2 jblks ahead of L2/L3
        for j in range(0, 8):
            emit_asm(j)
        emit_l2l3(0)
        for j in range(8, 12):
            emit_asm(j)
        emit_l2l3(1)
        for j in range(12, 16):
            emit_asm(j)
        emit_l2l3(2)
        emit_l2l3_tail(3)

    nc.compile()
    return nc


def _get_nc():
    if "nc" not in _CACHE:
        _CACHE["nc"] = _build_bass()
    return _CACHE["nc"]


def _ensure_device():
    """Probe the axon device; reset it if wedged."""
    if _CACHE.get("dev_ok"):
        return
    import jax
    import jax.numpy as jnp
    try:
        (jnp.zeros((8, 8)) + 1).block_until_ready()
    except Exception:
        import ctypes
        lib = ctypes.CDLL("/opt/axon/libaxon_pjrt.so")
        lib.axon_reset.restype = ctypes.c_int64
        jax.devices()
        lib.axon_reset()
        (jnp.zeros((8, 8)) + 1).block_until_ready()
    _CACHE["dev_ok"] = True


def _chan_split(doc_emb, W1, scale1, shift1, W2, q1max):
    """Rank h-channels by variance contribution to z2; light 512 go fp8.

    Returns (perm, s) with s the per-channel fp8 fold scale (1 for heavy).
    """
    A = ALPHA ** np.maximum(np.subtract.outer(np.arange(T), np.arange(T)), 0)
    A = np.where(np.subtract.outer(np.arange(T), np.arange(T)) >= 0, A, 0.0)
    A = A.astype(np.float32)
    U_l, U_c, U_r = W1[:, :DD], W1[:, DD:2 * DD], W1[:, 2 * DD:3 * DD]
    F = np.einsum('tk,bkd->btd', A, doc_emb)
    R = np.einsum('tk,bkd->btd', A.T, doc_emb)
    Gc = F @ U_c.T
    Gl = np.concatenate([np.zeros((B, 1, H4), np.float32),
                         F @ U_l.T], axis=1)[:, :T]
    Rp = np.concatenate([R, np.zeros((B, 1, DD), np.float32)], axis=1)
    Gr = Rp[:, 1:T + 1] @ U_r.T
    s_idx, e_idx = _CACHE.setdefault("cands", _cand_indices())
    sub = slice(0, None, 16)
    ss, ee = s_idx[sub], e_idx[sub]
    coef = (ALPHA ** (ee - ss + 1)).astype(np.float32)[None, :, None]
    Gcp = np.concatenate([np.zeros((B, 1, H4), np.float32), Gc], axis=1)
    z1s = Gl[:, ss] + Gc[:, ee] - coef * Gcp[:, ss] + Gr[:, ee]
    z1s = scale1 * z1s + shift1          # q1 term omitted: minor for ranking
    h2m = np.maximum(z1s, 0) ** 2
    contrib = h2m.mean(axis=(0, 1)) * (W2 ** 2).mean(axis=0)
    order = np.argsort(contrib)
    perm = np.concatenate([np.sort(order[512:]), np.sort(order[:512])])
    light = perm[512:]
    gmax = (np.abs(Gl).max(axis=(0, 1)) + (1 + ALPHA) * np.abs(Gc).max(axis=(0, 1))
            + np.abs(Gr).max(axis=(0, 1)))
    bound = np.abs(scale1) * (gmax + q1max) + np.abs(shift1) + 1e-6
    s = np.ones(H4, np.float32)
    s[light] = np.minimum(1.0, 240.0 / (1.1 * bound[light]))
    return perm, s


def _make_in_maps(inputs):
    import ml_dtypes
    doc_emb = np.asarray(inputs["doc_emb"], np.float32)
    query_emb = np.asarray(inputs["query_emb"], np.float32)
    W1 = np.asarray(inputs["W1"], np.float32)
    W2 = np.asarray(inputs["W2"], np.float32)
    W3 = np.asarray(inputs["W3"], np.float32)
    g1, b1, m1, v1 = (np.asarray(inputs[k], np.float32) for k in ("g1", "b1", "m1", "v1"))
    g2, b2, m2, v2 = (np.asarray(inputs[k], np.float32) for k in ("g2", "b2", "m2", "v2"))

    scale1 = g1 / np.sqrt(v1 + BN_EPS)
    shift1 = b1 - m1 * scale1
    scale2 = g2 / np.sqrt(v2 + BN_EPS)
    shift2 = b2 - m2 * scale2

    # channel permutation: heavy 512 first (bf16), light 512 last (fp8e4-DR);
    # fp8 overflow-guard scale s folded into bn1 stats and W2 columns.
    wv0 = ALPHA ** np.arange(LQ - 1, -1, -1, dtype=np.float32)
    q1max = np.abs(np.einsum('t,btd->bd', wv0, query_emb)
                   @ W1[:, 3 * DD:].T).max(axis=0)
    perm, s = _chan_split(doc_emb, W1, scale1, shift1, W2, q1max)
    W1 = W1[perm]
    W2 = W2[:, perm]
    scale1 = scale1[perm] * s[perm]
    shift1 = shift1[perm] * s[perm]
    W2 = W2 / s[perm][None, :]

    # host q-path: qf[b] = sum_t a^(Lq-1-t) q[b,t];  q1f[b] = scale1*(U_q qf) + shift1
    wv = ALPHA ** np.arange(LQ - 1, -1, -1, dtype=np.float32)
    qf = np.einsum('t,btd->bd', wv, query_emb)
    q1 = qf @ W1[:, 3 * DD:].T                     # [B, H4]
    q1f = scale1[None, :] * q1 + shift1[None, :]   # [B, H4]

    W1T = np.ascontiguousarray(W1.T)               # [1212, 1024]
    w1_h = np.zeros((6, 128, H4), ml_dtypes.bfloat16)
    w1s_h = np.zeros((3, 48, H4), ml_dtypes.bfloat16)
    for u in range(3):
        for kt, (k0, ksz) in enumerate(KT1):
            blk = W1T[u * DD + k0: u * DD + k0 + ksz]
            if kt < 2:
                w1_h[u * 2 + kt] = blk
            else:
                w1s_h[u] = blk
    w1_h = np.ascontiguousarray(w1_h.transpose(1, 0, 2))
    w1s_h = np.ascontiguousarray(w1s_h.transpose(1, 0, 2))

    W2T = W2.T                                     # [1024, 512], perm'd+scaled
    w2_h = np.ascontiguousarray(
        W2T[:512].reshape(4, 128, H2).transpose(1, 0, 2)).astype(ml_dtypes.bfloat16)
    w2f8_h = np.ascontiguousarray(
        np.clip(W2T[512:], -240, 240).reshape(2, 2, 128, H2)
        .transpose(2, 0, 1, 3)).astype(ml_dtypes.float8_e4m3)
    w3_h = np.ascontiguousarray(
        W3.T.reshape(NMT, 128, 2).transpose(1, 0, 2)).astype(ml_dtypes.bfloat16)

    bn2x = np.ascontiguousarray(
        np.stack([scale2.reshape(NMT, 128).T, shift2.reshape(NMT, 128).T],
                 axis=1))                           # [128, 2, NMT]

    amats = [_pack5(_build_amat(0), 2 * WIN, 0).astype(ml_dtypes.bfloat16),
             _pack5(_build_amat(403), 2 * WIN, 169).astype(ml_dtypes.bfloat16)]

    in_maps = []
    for core in range(N_CORES):
        b, half = core // 2, core % 2
        bn1x = np.ascontiguousarray(
            np.stack([scale1.reshape(NHT, 128).T,
                      q1f[b].reshape(NHT, 128).T], axis=1))   # [128, 2, NHT]
        in_maps.append({
            "doc": _pack5(doc_emb[b], DD, 169 * half).astype(ml_dtypes.bfloat16),
            "amat": amats[half],
            "w1": w1_h,
            "w1s": w1s_h,
            "w2": w2_h,
            "w2f8": w2f8_h,
            "w3": w3_h,
            "bn1x": bn1x,
            "bn2x": bn2x,
        })
    return in_maps


def _gather(results):
    s_idx, e_idx = _CACHE.setdefault("cands", _cand_indices())
    n = len(s_idx)
    j_idx = e_idx - s_idx
    half_idx = (s_idx >= 406).astype(np.int64)
    u_idx = s_idx - 403 * half_idx
    out = np.zeros((B, n, 2), np.float32)
    for b in range(B):
        both = np.stack([results[2 * b]["y"], results[2 * b + 1]["y"]])  # [2,2,16,NS]
        out[b] = both[half_idx, :, j_idx, u_idx]
    return out


def _run(inputs, trace=False):
    from concourse import bass_utils
    _ensure_device()
    nc = _get_nc()
    in_maps = _make_in_maps(inputs)
    res = bass_utils.run_bass_kernel_spmd(nc, in_maps,
                                          core_ids=list(range(N_CORES)),
                                          trace=trace)
    return _gather(res.results), res


def kernel(**inputs) -> np.ndarray:
    out, _ = _run(inputs, trace=False)
    return out
